# revision 2
# baseline (speedup 1.0000x reference)
"""Multi-head attention (B=8, N=1024, C=1024, H=16) on 8 TRN2 NeuronCores.

Sharding: batch-parallel — core c computes batch c end-to-end (12.9 GFLOP
per core, no collectives, output is a concat).

Design (v3):
  - All matmul operands are bf16 (f32 PSUM accumulation): same PE speed as
    f32r (1 cycle/row) but half the DMA bytes and SBUF footprint; end-to-end
    rel err ~7e-3 vs the 2e-2 gate.
  - x is transposed on the HOST: kernel input is xT [C, N] per core, so no
    PE transpose stage.
  - Software pipeline over head-pairs hp: qkv(hp+1) runs on the PE while
    exp(hp) drains on the Activation engine; S(u) matmul pairs interleave
    with attn@v of the previous unit so the PE never waits on exp.
  - Exp processes 2-bank PSUM tiles [128, 2x512]: one Act instruction per
    two S^T chunks.
  - PSUM: qkv/v 2 banks + S 2x2 banks + attn@v 2 banks = 8; the qkv pool
    closes after the last head-pair so the projection reuses its banks
    without waiting on the attention tail.
  - DMA issue order is tuned for the serial DMA backend: x first (gates
    everything), v/q/k weights next, W_proj + bias deferred into the loop.

Per-core algorithm (xT [C, N], weights full):
  q^T,k^T[hp] = (W_q|W_k cols).T @ xT  -> qk[hp] [128, 2, N] (2 heads/tile)
  v = xT.T @ W_v                       -> v_flat [128, 8, 16, 65] with ones
                                          column (softmax row sums for free)
  per unit (head, m-block): S^T chunk pairs -> exp [128,1024] -> P' bf16
  out'^T [65, 512] accumulated on PE; col 64 = row sums; normalize via
  reciprocal + DMA partition-broadcast (PE ones-broadcast for the last
  unit to shorten the projection critical path); y = oT.T @ W_proj + bias.
"""

import contextlib
import numpy as np

B, N, C, H, D = 8, 1024, 1024, 16, 64
HP = H // 2
SCALE = D ** -0.5
NCORES = 8
PCHUNKS = C // 128
TB = 512

_cached = {}


def _split_excess_waits(nc, max_waits=1):
    """walrus codegen limit: several lowered instruction structs (4-byte
    self-loading matmul S3_LW, drain CTRL_NO) carry only one sync-wait slot.
    Hoist excess waits onto InstEventSemaphore (2 waits each) just before
    the instruction on the same engine."""
    import concourse.mybir as mybir

    for func in nc.m.functions:
        for bb in func.blocks:
            insts = list(bb.instructions)
            out = []
            changed = False
            for inst in insts:
                si = inst.sync_info
                if (
                    si is not None
                    and not isinstance(inst, mybir.InstEventSemaphore)
                    and len(si.on_wait) > max_waits
                ):
                    waits = list(si.on_wait)
                    keep, excess = waits[:max_waits], waits[max_waits:]
                    for j in range(0, len(excess), 2):
                        ev = mybir.InstEventSemaphore(
                            name=nc.get_next_instruction_name(),
                            engine=inst.engine,
                            ins=[],
                            outs=[],
                            sync_info=mybir.SyncInfo(
                                on_wait=excess[j : j + 2], on_update=[]
                            ),
                        )
                        nc.register_instruction(ev)
                        out.append(ev)
                    si.on_wait = keep
                    inst.sync_info = si
                    changed = True
                out.append(inst)
            if changed:
                bb.instructions = out


def _build(n_rep=1):
    import concourse.bass as bass
    import concourse.mybir as mybir
    import concourse.tile as tile

    f32 = mybir.dt.float32
    bf16 = mybir.dt.bfloat16
    Exp = mybir.ActivationFunctionType.Exp

    nc = bass.Bass()
    xT = nc.declare_dram_parameter("xT", [C, N], bf16, isOutput=False)
    wqkv = nc.declare_dram_parameter("W_qkv", [C, 3 * C], bf16, isOutput=False)
    wproj = nc.declare_dram_parameter("W_proj", [C, C], bf16, isOutput=False)
    bproj = nc.declare_dram_parameter("b_proj", [C], f32, isOutput=False)
    ones_in = nc.declare_dram_parameter("ones64", [128, D], bf16, isOutput=False)
    y = nc.declare_dram_parameter("y", [N, C], f32, isOutput=True)

    wqkv_t = wqkv[:].rearrange("(cc p) m -> p cc m", p=128)  # [128, 8, 3C]
    wproj_t = wproj[:].rearrange("(cc p) m -> p cc m", p=128)  # [128, 8, C]
    bproj_bcast = bass.AP(tensor=bproj, offset=0, ap=[[0, 128], [1, C]])

    with tile.TileContext(nc) as tc:
        with contextlib.ExitStack() as ctx:
            consts = ctx.enter_context(tc.tile_pool(name="consts", bufs=1))
            ones64 = consts.tile([128, D], bf16)
            nc.scalar.dma_start(out=ones64, in_=ones_in[:])
            b_bc = consts.tile([128, C], f32)

            for rep in range(n_rep):
              with contextlib.ExitStack() as rctx:
                ep = rctx.enter_context
                xT_p = ep(tc.tile_pool(name=f"xT_r{rep}", bufs=1))
                v_p = ep(tc.tile_pool(name=f"v_r{rep}", bufs=1))
                oT_p = ep(tc.tile_pool(name=f"oT_r{rep}", bufs=1))
                wp_p = ep(tc.tile_pool(name=f"wp_r{rep}", bufs=1))
                wqk_p = ep(tc.tile_pool(name=f"wqk_r{rep}", bufs=2))
                qk_p = ep(tc.tile_pool(name=f"qk_r{rep}", bufs=2))
                e2_p = ep(tc.tile_pool(name=f"e2_r{rep}", bufs=8))
                o65_p = ep(tc.tile_pool(name=f"o65_r{rep}", bufs=4))
                inv_p = ep(tc.tile_pool(name=f"inv_r{rep}", bufs=4))
                invd_p = ep(tc.tile_pool(name=f"invd_r{rep}", bufs=4, space="DRAM"))
                invb_p = ep(tc.tile_pool(name=f"invb_r{rep}", bufs=4))
                psst_p = ep(tc.tile_pool(name=f"psst_r{rep}", bufs=4, space="PSUM"))
                pso_p = ep(tc.tile_pool(name=f"pso_r{rep}", bufs=2, space="PSUM"))
                psq_ctx = contextlib.ExitStack()
                psq_p = psq_ctx.enter_context(
                    tc.tile_pool(name=f"psq_r{rep}", bufs=2, space="PSUM")
                )

                xTs = [
                    xT_p.tile([128, N], bf16, name=f"xT{c}_r{rep}", tag=f"xT{c}")
                    for c in range(PCHUNKS)
                ]
                v_flat = v_p.tile(
                    [128, PCHUNKS, H, D + 1], bf16, name=f"v_r{rep}", tag="v"
                )
                v_ext = [v_flat[:, t] for t in range(PCHUNKS)]
                oT = [
                    oT_p.tile([128, N], bf16, name=f"oT{i}_r{rep}", tag=f"oT{i}")
                    for i in range(HP)
                ]
                wp = wp_p.tile([128, PCHUNKS, C], bf16, name=f"wp_r{rep}", tag="wp")

                def dma_wqk(hp):
                    wq = wqk_p.tile(
                        [128, PCHUNKS, 128], bf16, name=f"wq{hp}_r{rep}", tag="wq"
                    )
                    nc.scalar.dma_start(
                        out=wq, in_=wqkv_t[:, :, hp * 128 : (hp + 1) * 128]
                    )
                    wk = wqk_p.tile(
                        [128, PCHUNKS, 128], bf16, name=f"wk{hp}_r{rep}", tag="wk"
                    )
                    nc.scalar.dma_start(
                        out=wk, in_=wqkv_t[:, :, C + hp * 128 : C + (hp + 1) * 128]
                    )
                    return wq, wk

                # DMA order: x (gates everything) -> q/k weights for hp0 ->
                # v weights -> ones columns.
                for c in range(PCHUNKS):
                    nc.sync.dma_start(out=xTs[c], in_=xT[c * 128 : (c + 1) * 128, :])
                wqk_tiles = {0: dma_wqk(0)}

                qk = {}

                def qkv_mm(hp):
                    """q^T,k^T for head-pair hp -> qk[hp] [128, 2, N]."""
                    wq, wk = wqk_tiles.pop(hp)
                    qk[hp] = qk_p.tile(
                        [128, 2, N], bf16, name=f"qk{hp}_r{rep}", tag="qk"
                    )
                    for tb in range(N // TB):
                        for qi, w in ((0, wq), (1, wk)):
                            p = psq_p.tile([128, TB], f32, name=f"pq_r{rep}", tag="pq")
                            for c in range(PCHUNKS):
                                nc.tensor.matmul(
                                    p,
                                    w[:, c, :],
                                    xTs[c][:, tb * TB : (tb + 1) * TB],
                                    start=(c == 0),
                                    stop=(c == PCHUNKS - 1),
                                )
                            nc.vector.tensor_copy(
                                qk[hp][:, qi, tb * TB : (tb + 1) * TB], p
                            )

                # ---------------- qkv(hp0), then v = x @ W_v ----------------
                with tc.tile_pool(name=f"wv_r{rep}", bufs=2) as wv_p:
                    wvs = []
                    for vb in range(2):
                        wv = wv_p.tile(
                            [128, PCHUNKS, TB], bf16, name=f"wv{vb}_r{rep}", tag="wv"
                        )
                        nc.scalar.dma_start(
                            out=wv,
                            in_=wqkv_t[:, :, 2 * C + vb * TB : 2 * C + (vb + 1) * TB],
                        )
                        wvs.append(wv)
                    for t in range(PCHUNKS):
                        nc.vector.tensor_copy(
                            v_ext[t][:, :, D : D + 1], ones64[:, 0:H, None]
                        )

                    qkv_mm(0)

                    for vb in range(2):
                        wv = wvs[vb]
                        for t in range(PCHUNKS):
                            pv = psq_p.tile([128, TB], f32, name=f"pv_r{rep}", tag="pq")
                            for c in range(PCHUNKS):
                                nc.tensor.matmul(
                                    pv,
                                    xTs[c][:, t * 128 : (t + 1) * 128],
                                    wv[:, c, :],
                                    start=(c == 0),
                                    stop=(c == PCHUNKS - 1),
                                )
                            nc.vector.tensor_copy(
                                v_ext[t][:, vb * 8 : (vb + 1) * 8, 0:D],
                                pv.rearrange("p (h d) -> p h d", h=8),
                            )

                # ---------------- attention pipeline ----------------
                def S_pair(hp, hh, mb, tp):
                    """Two S^T chunk matmuls, one exp each (single-bank
                    PSUM tiles) -> P' bf16."""
                    e2 = e2_p.tile([128, 2, TB], bf16, name=f"e2_r{rep}", tag="e2")
                    for i in (0, 1):
                        t = 2 * tp + i
                        ps1 = psst_p.tile([128, TB], f32, name=f"ps1_r{rep}", tag="ps1")
                        nc.tensor.matmul(
                            ps1,
                            qk[hp][64 * hh : 64 * hh + 64, 1, t * 128 : (t + 1) * 128],
                            qk[hp][64 * hh : 64 * hh + 64, 0, mb * TB : (mb + 1) * TB],
                            start=True,
                            stop=True,
                        )
                        nc.scalar.activation(e2[:, i, :], ps1, Exp, scale=SCALE)
                    return e2

                def av_mms(po, hp, hh, e2t, tp):
                    h = 2 * hp + hh
                    for i in (0, 1):
                        t = 2 * tp + i
                        nc.tensor.matmul(
                            po,
                            v_ext[t][:, h, :],
                            e2t[:, i, :],
                            start=(t == 0),
                            stop=(t == PCHUNKS - 1),
                        )

                def av_norm(po, hp, hh, mb):
                    """Copy PSUM out, reciprocal of sums, DMA partition
                    broadcast, normalize into oT."""
                    o65 = o65_p.tile([D + 1, TB], f32, name=f"o65_r{rep}", tag="o65")
                    nc.vector.tensor_copy(o65, po)
                    inv = inv_p.tile([1, TB], f32, name=f"inv_r{rep}", tag="inv")
                    nc.vector.reciprocal(inv, o65[D : D + 1, :])
                    dinv = invd_p.tile([1, TB], f32, name=f"dinv_r{rep}", tag="dinv")
                    nc.sync.dma_start(out=dinv, in_=inv)
                    ib = invb_p.tile([D, TB], f32, name=f"invb_r{rep}", tag="invb")
                    nc.sync.dma_start(
                        out=ib,
                        in_=bass.AP(
                            tensor=dinv.tensor,
                            offset=dinv.offset,
                            ap=[[0, D]] + list(dinv.ap)[1:],
                        ),
                    )
                    nc.vector.tensor_mul(
                        oT[hp][hh * D : (hh + 1) * D, mb * TB : (mb + 1) * TB],
                        o65[0:D, :],
                        ib,
                    )

                UNITS = [(hh, mb) for hh in range(2) for mb in range(2)]
                pending = None

                for hp in range(HP):
                    if hp + 1 < HP:
                        wqk_tiles[hp + 1] = dma_wqk(hp + 1)
                    if hp < 2:
                        half = C // 2
                        nc.sync.dma_start(
                            out=wp[:, :, hp * half : (hp + 1) * half],
                            in_=wproj_t[:, :, hp * half : (hp + 1) * half],
                        )
                    if hp == 2:
                        nc.sync.dma_start(out=b_bc, in_=bproj_bcast)
                    for hh, mb in UNITS:
                        po = pso_p.tile([D + 1, TB], f32, name=f"po_r{rep}", tag="po")
                        e2ts = []
                        for tp in range(PCHUNKS // 2):
                            e2ts.append(S_pair(hp, hh, mb, tp))
                            if pending is not None:
                                av_mms(
                                    pending[0], pending[1], pending[2],
                                    pending[4][tp], tp,
                                )
                        if pending is not None:
                            av_norm(pending[0], pending[1], pending[2], pending[3])
                        pending = (po, hp, hh, mb, e2ts)
                    if hp + 1 < HP:
                        qkv_mm(hp + 1)

                psq_ctx.close()
                # drain last unit
                po, hp, hh, mb, e2ts = pending
                for tp in range(PCHUNKS // 2):
                    av_mms(po, hp, hh, e2ts[tp], tp)
                av_norm(po, hp, hh, mb)

                # ---------------- output projection ----------------
                with (
                    tc.tile_pool(name=f"ysb_r{rep}", bufs=3) as ysb_p,
                    tc.tile_pool(name=f"psy_r{rep}", bufs=2, space="PSUM") as psy_p,
                ):
                    # token blocks 0-3 first: their oT inputs are ready well
                    # before the last unit's (blocks 4-7) normalize lands.
                    for mc_grp in (range(0, 4), range(4, 8)):
                        for cb in range(C // TB):
                            for mc in mc_grp:
                                py = psy_p.tile(
                                    [128, TB], f32, name=f"py_r{rep}", tag="py"
                                )
                                for hp_ in range(HP):
                                    nc.tensor.matmul(
                                        py,
                                        oT[hp_][:, mc * 128 : (mc + 1) * 128],
                                        wp[:, hp_, cb * TB : (cb + 1) * TB],
                                        start=(hp_ == 0),
                                        stop=(hp_ == HP - 1),
                                    )
                                ys = ysb_p.tile(
                                    [128, TB], f32, name=f"ys_r{rep}", tag="ys"
                                )
                                nc.vector.tensor_add(
                                    ys, py, b_bc[:, cb * TB : (cb + 1) * TB]
                                )
                                nc.sync.dma_start(
                                    out=y[
                                        mc * 128 : (mc + 1) * 128,
                                        cb * TB : (cb + 1) * TB,
                                    ],
                                    in_=ys,
                                )

    _split_excess_waits(nc)
    nc.finalize()
    return nc


def _get_nc(n_rep=1):
    key = f"nc{n_rep}"
    if key not in _cached:
        _cached[key] = _build(n_rep)
    return _cached[key]


def make_in_maps(x, W_qkv, W_proj, b_proj):
    import ml_dtypes

    bf16 = ml_dtypes.bfloat16
    x = np.asarray(x, dtype=np.float32)
    W_qkv = np.ascontiguousarray(np.asarray(W_qkv, dtype=np.float32).astype(bf16))
    W_proj = np.ascontiguousarray(np.asarray(W_proj, dtype=np.float32).astype(bf16))
    b_proj = np.ascontiguousarray(np.asarray(b_proj, dtype=np.float32))
    ones64 = np.ones((128, D), dtype=bf16)
    return [
        {
            "xT": np.ascontiguousarray(x[c].T.astype(bf16)),
            "W_qkv": W_qkv,
            "W_proj": W_proj,
            "b_proj": b_proj,
            "ones64": ones64,
        }
        for c in range(NCORES)
    ]


def kernel(x, W_qkv, W_proj, b_proj, **_ignored):
    from concourse.bass_utils import run_bass_kernel_spmd

    nc = _get_nc()
    in_maps = make_in_maps(x, W_qkv, W_proj, b_proj)
    try:
        res = run_bass_kernel_spmd(nc, in_maps, core_ids=list(range(NCORES)))
    except Exception:
        # transient device errors (e.g. NRT_EXEC_UNIT_UNRECOVERABLE) recover
        # on re-dispatch
        import time as _time

        _time.sleep(10)
        res = run_bass_kernel_spmd(nc, in_maps, core_ids=list(range(NCORES)))
    out = np.stack([res.results[c]["y"] for c in range(NCORES)], axis=0)
    return out.astype(np.float32)
